# revision 1
# baseline (speedup 1.0000x reference)
"""ExpertScatter TRN2 kernel.

reference semantics:
    X = einsum('bekj,eji->beki', Y, W)          # per-head projection
    out[b] = zeros([T, I]); out[b, Ind[b,e,k]] += X[b,e,k]

Strategy (data-parallel over batch, 1 batch per NeuronCore):
  Phase A: per head e, matmul X_chunk[128 rows, 1024] = Yt_chunk.T @ W[e]
           (fp16 operands by default; float32r available = full PE rate
           with fp32 data), write X to an internal HBM staging buffer in
           natural row order (fp16 halves the round-trip traffic).
  Host precomputes a global sort of the 16384 rows of each batch by target
  slot, padded to a fixed PT rows per 128-slot output tile.
  Phase B: per output tile (128 slots), dma_gather the contributing rows
           (PT of them) into SBUF, build one-hot selection matrices on DVE
           (is_equal against a column-iota constant), and accumulate
           out_tile = sum_g onehot_g.T @ Xrows_g in PSUM. One DMA per tile
           writes the finished [128, 1024] block of the output.

All shapes/counts are identical across cores (SPMD); per-core data
differences live entirely in the input tensors (Yt, gather indices,
relative-column tables).
"""

import os

import numpy as np

import concourse.bacc as bacc
import concourse.mybir as mybir
import concourse.tile as tile
from concourse.bass_utils import run_bass_kernel_spmd

# Problem constants (hardcoded per harness contract).
B = 8
HEADS = 16
K = 1024
HEAD_DIM = 128
OUT_DIM = 1024
T_SLOTS = 4096

R = HEADS * K            # rows per batch = 16384
NT = T_SLOTS // 128      # output tiles per batch = 32
PT = 640                 # gather buffer rows per output tile (5 groups)
NG = PT // 128           # row groups (matmuls) per output tile = 5
NCORES = 8

F32 = mybir.dt.float32
F32R = mybir.dt.float32r
BF16 = mybir.dt.bfloat16
FP16 = mybir.dt.float16
I16 = mybir.dt.int16

# Projection matmul dtype: "f32r" (full-rate fp32), "f32" (4x slower),
# or "fp16" (halves Y/W traffic, ~2x err).
MM_DTYPE = os.environ.get("ES_MM_DTYPE", "fp16")
MM_F32R = MM_DTYPE == "f32r"
# X staging / scatter dtype: "fp16", "bf16", "f32r", or "f32".
X_DTYPE = os.environ.get("ES_X_DTYPE", "fp16")
# Debug: which phases to emit ("AB", "A", "B").
PHASES = os.environ.get("ES_PHASES", "AB")
# Scheduling knobs.
GBUFS = int(os.environ.get("ES_GBUFS", "4"))
XBUFS = int(os.environ.get("ES_XBUFS", "6"))
WSPLIT = os.environ.get("ES_WSPLIT", "1") == "1"
BARRIER = os.environ.get("ES_BARRIER", "0") == "1"
# Write the output in fp16 (host casts back to f32): halves out traffic.
OUT_FP16 = os.environ.get("ES_OUT_FP16", "1") == "1"
# Alternate PSUM->SBUF copies between DVE and ACT.
ALT_COPY = os.environ.get("ES_ALT_COPY", "1") == "1"
# Trailing -1 index padding (skipped by Q7 -> less gather traffic). Found
# unreliable on HW at full scale (intermittent NRT faults) -> default off.
EXACT_CNT = os.environ.get("ES_EXACT_CNT", "0") == "1"

_cache = {}


def _build_program(mdt, sdt, gnum):
    """mdt: projection matmul dtype; sdt: X staging + scatter dtype;
    gnum: gathered positions per tile (<= PT; rest is never read thanks to
    the one-hot sentinel, but must hold finite values)."""
    nc = bacc.Bacc("TRN2", target_bir_lowering=False, debug=False,
                   num_devices=NCORES)

    yt = nc.dram_tensor("yt", [HEAD_DIM, R], mdt, kind="ExternalInput").ap()
    w = nc.dram_tensor("w", [HEAD_DIM, HEADS * OUT_DIM], mdt,
                       kind="ExternalInput").ap()
    gidx = nc.dram_tensor("gidx", [128, NT * (PT // 16)], I16,
                          kind="ExternalInput").ap()
    relc = nc.dram_tensor("relc", [128, NT * NG], F32,
                          kind="ExternalInput").ap()
    cols = nc.dram_tensor("cols", [128, 128], F32, kind="ExternalInput").ap()
    odt = FP16 if OUT_FP16 else F32
    out = nc.dram_tensor("out", [T_SLOTS, OUT_DIM], odt,
                         kind="ExternalOutput").ap()
    xnat = nc.dram_tensor("xnat", [R, OUT_DIM], sdt).ap()

    with tile.TileContext(nc) as tc:
        with (
            tc.tile_pool(name="const", bufs=1) as cpool,
            tc.tile_pool(name="yhead",
                         bufs=int(os.environ.get("ES_YBUFS", "2"))) as ypool,
            tc.tile_pool(name="xchunk", bufs=XBUFS) as xpool,
            tc.tile_pool(name="gather", bufs=GBUFS) as gpool,
            tc.tile_pool(name="onehot",
                         bufs=int(os.environ.get("ES_OHBUFS", "4"))) as ohpool,
            tc.tile_pool(name="otile",
                         bufs=int(os.environ.get("ES_OBUFS", "4"))) as opool,
        ):
            w_sb = cpool.tile([128, HEADS * OUT_DIM], mdt, tag="w")
            if WSPLIT:
                for e in range(HEADS):
                    nc.sync.dma_start(
                        out=w_sb[:, e * OUT_DIM:(e + 1) * OUT_DIM],
                        in_=w[:, e * OUT_DIM:(e + 1) * OUT_DIM])
            else:
                nc.sync.dma_start(out=w_sb[:], in_=w[:])
            gidx_sb = cpool.tile([128, NT * (PT // 16)], I16, tag="gidx")
            nc.sync.dma_start(out=gidx_sb[:], in_=gidx[:])
            relc_sb = cpool.tile([128, NT * NG], F32, tag="relc")
            nc.sync.dma_start(out=relc_sb[:], in_=relc[:])
            cols_sb = cpool.tile([128, 128], F32, tag="cols")
            nc.sync.dma_start(out=cols_sb[:], in_=cols[:])

            # ---- Phase A: projection, X written to HBM in natural order --
            pa_ctx = tc.tile_pool(name="psumA",
                                  bufs=int(os.environ.get("ES_PABUFS", "2")),
                                  space="PSUM")
            pspool = pa_ctx.__enter__()
            for e in range(HEADS if "A" in PHASES else 0):
                yt_e = ypool.tile([128, K], mdt, tag="yt")
                nc.sync.dma_start(out=yt_e[:], in_=yt[:, e * K:(e + 1) * K])
                for rc in range(K // 128):
                    px = pspool.tile([128, OUT_DIM], F32, tag="pa")
                    lhsT = yt_e[:, rc * 128:(rc + 1) * 128]
                    for h in range(2):
                        nc.tensor.matmul(
                            out=px[:, h * 512:(h + 1) * 512],
                            lhsT=lhsT,
                            rhs=w_sb[:, e * OUT_DIM + h * 512:
                                     e * OUT_DIM + (h + 1) * 512],
                            start=True, stop=True,
                        )
                    xc = xpool.tile([128, OUT_DIM], sdt, tag="xc")
                    if ALT_COPY and rc % 2 == 1:
                        nc.scalar.copy(out=xc[:], in_=px[:])
                    else:
                        nc.vector.tensor_copy(out=xc[:], in_=px[:])
                    row0 = (e * (K // 128) + rc) * 128
                    xeng = (nc.scalar if os.environ.get("ES_DMAALT", "0") == "1"
                            and rc % 2 == 0 else nc.sync)
                    xeng.dma_start(out=xnat[row0:row0 + 128, :], in_=xc[:])

            pa_ctx.__exit__(None, None, None)

            # Fence: every gather below reads rows written above.
            if BARRIER and "A" in PHASES and "B" in PHASES:
                tc.strict_bb_all_engine_barrier()
            pb_ctx = tc.tile_pool(name="psumB",
                                  bufs=int(os.environ.get("ES_PBBUFS", "2")),
                                  space="PSUM")
            pspool = pb_ctx.__enter__()

            # ---- Phase B: gather sorted rows per tile, one-hot matmul ----
            splitg = os.environ.get("ES_SPLITG", "1") == "1"
            for t in range(NT if "B" in PHASES else 0):
                g = gpool.tile([128, NG, OUT_DIM], sdt, tag="g")
                if EXACT_CNT and t < GBUFS:
                    # With -1 skip-padding, unwritten positions vary per
                    # tile; scrub whole fresh slots once so unread regions
                    # hold finite values (one-hot sentinel zeroes them).
                    nc.gpsimd.memset(g[:], 0.0)
                elif gnum < PT and t < GBUFS:
                    # Positions gnum..PT are never gathered; scrub the
                    # fresh SBUF slots once so the unread region holds
                    # finite values (one-hot sentinel zeroes them out).
                    lastp = (gnum // 128) * 128
                    nc.gpsimd.memset(g[gnum - lastp:, NG - 1, :], 0.0)
                gq = (t % 2) if os.environ.get("ES_GQALT", "0") == "1" else 0
                if splitg:
                    cut = int(os.environ.get("ES_GCUT", "384"))
                    nc.gpsimd.dma_gather(
                        out_ap=g[:, 0:cut // 128, :],
                        in_ap=xnat[:],
                        idxs_ap=gidx_sb[:, t * (PT // 16):
                                        t * (PT // 16) + cut // 16],
                        num_idxs=cut, num_idxs_reg=cut, elem_size=OUT_DIM,
                        queue_num=gq,
                    )
                    nc.gpsimd.dma_gather(
                        out_ap=g[:, cut // 128:NG, :],
                        in_ap=xnat[:],
                        idxs_ap=gidx_sb[:, t * (PT // 16) + cut // 16:
                                        t * (PT // 16) + gnum // 16],
                        num_idxs=gnum - cut, num_idxs_reg=gnum - cut,
                        elem_size=OUT_DIM, queue_num=gq,
                    )
                else:
                    nc.gpsimd.dma_gather(
                        out_ap=g[:],
                        in_ap=xnat[:],
                        idxs_ap=gidx_sb[:, t * (PT // 16):
                                        t * (PT // 16) + gnum // 16],
                        num_idxs=gnum,
                        num_idxs_reg=gnum,
                        elem_size=OUT_DIM,
                    )
                pt = pspool.tile([128, OUT_DIM], F32, tag="pb")
                for gi in range(NG):
                    oh = ohpool.tile([128, 128], sdt, tag="oh")
                    c = t * NG + gi
                    nc.vector.tensor_tensor(
                        out=oh[:],
                        in0=relc_sb[:, c:c + 1].to_broadcast([128, 128]),
                        in1=cols_sb[:],
                        op=mybir.AluOpType.is_equal,
                    )
                    for h in range(2):
                        nc.tensor.matmul(
                            out=pt[:, h * 512:(h + 1) * 512],
                            lhsT=oh[:],
                            rhs=g[:, gi, h * 512:(h + 1) * 512],
                            start=(gi == 0), stop=(gi == NG - 1),
                        )
                ot = opool.tile([128, OUT_DIM], odt, tag="ot")
                if ALT_COPY and t % 2 == 1:
                    nc.scalar.copy(out=ot[:], in_=pt[:])
                else:
                    nc.vector.tensor_copy(out=ot[:], in_=pt[:])
                nc.sync.dma_start(out=out[t * 128:(t + 1) * 128, :], in_=ot[:])
            pb_ctx.__exit__(None, None, None)

    nc.compile()
    return nc


def _get_program(gnum=576):
    mdt = {"f32r": F32R, "f32": F32, "fp16": FP16, "bf16": BF16}[MM_DTYPE]
    sdt = {"f32r": F32R if MM_F32R else F32, "f32": F32,
           "bf16": BF16, "fp16": FP16}[X_DTYPE]
    key = (MM_DTYPE, X_DTYPE, PHASES, GBUFS, XBUFS, WSPLIT, BARRIER,
           ALT_COPY, EXACT_CNT, OUT_FP16, gnum,
           os.environ.get("ES_SPLITG", "1"),
           os.environ.get("ES_OBUFS", "4"), os.environ.get("ES_YBUFS", "2"),
           os.environ.get("ES_PABUFS", "2"), os.environ.get("ES_PBBUFS", "2"))
    if key not in _cache:
        _cache[key] = _build_program(mdt, sdt, gnum)
    return _cache[key]


def _prep_core_inputs(Yb, Indb):
    """Host-side prep for one batch: transpose Y, sort rows by slot,
    build padded gather-index and relative-column tables."""
    yt = np.ascontiguousarray(
        Yb.transpose(2, 0, 1).reshape(HEAD_DIM, R)).astype(np.float32)
    ind = Indb.reshape(R).astype(np.int64)
    order = np.argsort(ind, kind="stable")
    sind = ind[order]
    counts = np.bincount(sind // 128, minlength=NT)
    assert counts.max() <= PT, f"tile overflow: {counts.max()} > {PT}"
    _prep_core_inputs.max_count = max(
        getattr(_prep_core_inputs, "max_count", 0), int(counts.max()))
    pad = -1 if EXACT_CNT else 0
    gidx = np.full((NT, PT), pad, dtype=np.int16)
    relc = np.full((NT, PT), -1000.0, dtype=np.float32)
    pos = 0
    for t in range(NT):
        c = counts[t]
        gidx[t, :c] = order[pos:pos + c]
        relc[t, :c] = (sind[pos:pos + c] - t * 128).astype(np.float32)
        pos += c
    # dma_gather index layout: position p -> (partition p%16, col p//16),
    # and the 16-partition block replicated across all 8 Q7 core groups.
    blk = np.concatenate(
        [gidx[t].reshape(PT // 16, 16).T for t in range(NT)], axis=1)
    gidx_sb = np.ascontiguousarray(np.tile(blk, (8, 1)), dtype=np.int16)
    # one-hot layout: position p -> (partition p%128, group p//128)
    relc_sb = np.concatenate(
        [relc[t].reshape(NG, 128).T for t in range(NT)], axis=1)
    relc_sb = np.ascontiguousarray(relc_sb, dtype=np.float32)
    return yt, gidx_sb, relc_sb


def kernel(Y, Ind, T, W):
    Y = np.asarray(Y, dtype=np.float32)
    Ind = np.asarray(Ind)
    W = np.asarray(W, dtype=np.float32)
    assert int(T) == T_SLOTS and Y.shape == (B, HEADS, K, HEAD_DIM)

    if MM_DTYPE == "fp16":
        np_mdt = np.float16
    elif MM_DTYPE == "bf16":
        import ml_dtypes
        np_mdt = ml_dtypes.bfloat16
    else:
        np_mdt = np.float32
    w_in = np.ascontiguousarray(
        W.transpose(1, 0, 2).reshape(HEAD_DIM, HEADS * OUT_DIM)
    ).astype(np_mdt)
    cols_in = np.broadcast_to(
        np.arange(128, dtype=np.float32)[None, :], (128, 128)).copy()

    _prep_core_inputs.max_count = 0
    in_maps = []
    for b in range(B):
        yt, gidx_sb, relc_sb = _prep_core_inputs(Y[b], Ind[b])
        in_maps.append({
            "yt": yt.astype(np_mdt), "w": w_in, "gidx": gidx_sb,
            "relc": relc_sb, "cols": cols_in,
        })
    gnum = 576 if _prep_core_inputs.max_count <= 576 else PT
    nc = _get_program(gnum)

    # The first execution of a freshly compiled NEFF occasionally wedges a
    # core (NRT_EXEC_UNIT_UNRECOVERABLE); a retry on a fresh execute has
    # been observed to recover.
    last_exc = None
    for attempt in range(3):
        try:
            res = run_bass_kernel_spmd(
                nc, in_maps, core_ids=list(range(NCORES)),
                trace=os.environ.get("ES_TRACE", "0") == "1",
            )
            break
        except Exception as exc:  # noqa: BLE001 - device flake, retry
            last_exc = exc
            import time as _time
            _time.sleep(2.0)
    else:
        raise last_exc
    kernel.last_results = res
    out = np.stack([res.results[b]["out"] for b in range(B)], axis=0)
    return out.astype(np.float32)



# revision 15
# speedup vs baseline: 1.1350x; 1.1350x over previous
"""ExpertScatter TRN2 kernel.

reference semantics:
    X = einsum('bekj,eji->beki', Y, W)          # per-head projection
    out[b] = zeros([T, I]); out[b, Ind[b,e,k]] += X[b,e,k]

Strategy (data-parallel over batch, 1 batch per NeuronCore):
  Phase A: per head e, matmul X_chunk[128 rows, 1024] = Yt_chunk.T @ W[e]
           (fp16 operands by default; float32r available = full PE rate
           with fp32 data), write X to an internal HBM staging buffer in
           natural row order (fp16 halves the round-trip traffic).
  Host precomputes a global sort of the 16384 rows of each batch by target
  slot, padded to a fixed PT rows per 128-slot output tile.
  Phase B: per output tile (128 slots), dma_gather the contributing rows
           (PT of them) into SBUF, build one-hot selection matrices on DVE
           (is_equal against a column-iota constant), and accumulate
           out_tile = sum_g onehot_g.T @ Xrows_g in PSUM. One DMA per tile
           writes the finished [128, 1024] block of the output.

All shapes/counts are identical across cores (SPMD); per-core data
differences live entirely in the input tensors (Yt, gather indices,
relative-column tables).
"""

import os

import numpy as np

import concourse.bacc as bacc
import concourse.mybir as mybir
import concourse.tile as tile
from concourse.bass_utils import run_bass_kernel_spmd

# Problem constants (hardcoded per harness contract).
B = 8
HEADS = 16
K = 1024
HEAD_DIM = 128
OUT_DIM = 1024
T_SLOTS = 4096

R = HEADS * K            # rows per batch = 16384
NT = T_SLOTS // 128      # output tiles per batch = 32
PT = 640                 # gather buffer rows per output tile (5 groups)
NG = PT // 128           # row groups (matmuls) per output tile = 5
NCORES = 8

F32 = mybir.dt.float32
F32R = mybir.dt.float32r
BF16 = mybir.dt.bfloat16
FP16 = mybir.dt.float16
I16 = mybir.dt.int16

# Projection matmul dtype: "f32r" (full-rate fp32), "f32" (4x slower),
# or "fp16" (halves Y/W traffic, ~2x err).
MM_DTYPE = os.environ.get("ES_MM_DTYPE", "fp16")
MM_F32R = MM_DTYPE == "f32r"
# X staging / scatter dtype: "fp16", "bf16", "f32r", or "f32".
X_DTYPE = os.environ.get("ES_X_DTYPE", "fp16")
# Debug: which phases to emit ("AB", "A", "B").
PHASES = os.environ.get("ES_PHASES", "AB")
# Scheduling knobs.
GBUFS = int(os.environ.get("ES_GBUFS", "4"))
XBUFS = int(os.environ.get("ES_XBUFS", "8"))
WSPLIT = os.environ.get("ES_WSPLIT", "1") == "1"
BARRIER = os.environ.get("ES_BARRIER", "0") == "1"
# Write the output in fp16 (host casts back to f32): halves out traffic.
OUT_FP16 = os.environ.get("ES_OUT_FP16", "1") == "1"
# Alternate PSUM->SBUF copies between DVE and ACT.
ALT_COPY = os.environ.get("ES_ALT_COPY", "1") == "1"
# Trailing -1 index padding (skipped by Q7 -> less gather traffic). Found
# unreliable on HW at full scale (intermittent NRT faults) -> default off.
EXACT_CNT = os.environ.get("ES_EXACT_CNT", "0") == "1"

_cache = {}


def _build_program(mdt, sdt, gnums):
    """mdt: projection matmul dtype; sdt: X staging + scatter dtype;
    gnums: per-tile gathered positions (544..PT, x16; the ungathered tail
    is never read thanks to the one-hot sentinel, but must hold finite
    values -- the tail region past min(gnums) is scrubbed once per slot)."""
    nc = bacc.Bacc("TRN2", target_bir_lowering=False, debug=False,
                   num_devices=NCORES)

    yt = nc.dram_tensor("yt", [HEAD_DIM, R], mdt, kind="ExternalInput").ap()
    w = nc.dram_tensor("w", [HEAD_DIM, HEADS * OUT_DIM], mdt,
                       kind="ExternalInput").ap()
    gidx = nc.dram_tensor("gidx", [128, NT * (PT // 16)], I16,
                          kind="ExternalInput").ap()
    relc = nc.dram_tensor("relc", [128, NT * NG], F32,
                          kind="ExternalInput").ap()
    cols = nc.dram_tensor("cols", [128, 128], F32, kind="ExternalInput").ap()
    odt = FP16 if OUT_FP16 else F32
    out = nc.dram_tensor("out", [T_SLOTS, OUT_DIM], odt,
                         kind="ExternalOutput").ap()
    xnat = nc.dram_tensor("xnat", [R, OUT_DIM], sdt).ap()

    with tile.TileContext(nc) as tc:
        with (
            tc.tile_pool(name="const", bufs=1) as cpool,
            tc.tile_pool(name="yhead",
                         bufs=int(os.environ.get("ES_YBUFS", "4"))) as ypool,
            tc.tile_pool(name="xchunk", bufs=XBUFS) as xpool,
            tc.tile_pool(name="gather", bufs=GBUFS) as gpool,
            tc.tile_pool(name="onehot",
                         bufs=int(os.environ.get("ES_OHBUFS", "4"))) as ohpool,
            tc.tile_pool(name="otile",
                         bufs=int(os.environ.get("ES_OBUFS", "4"))) as opool,
        ):
            w_sb = cpool.tile([128, HEADS * OUT_DIM], mdt, tag="w")
            gidx_sb = cpool.tile([128, NT * (PT // 16)], I16, tag="gidx")
            relc_sb = cpool.tile([128, NT * NG], F32, tag="relc")
            cols_sb = cpool.tile([128, 128], F32, tag="cols")

            def load_w(e):
                nc.sync.dma_start(
                    out=w_sb[:, e * OUT_DIM:(e + 1) * OUT_DIM],
                    in_=w[:, e * OUT_DIM:(e + 1) * OUT_DIM])

            if not WSPLIT:
                nc.sync.dma_start(out=w_sb[:], in_=w[:])

            # ---- Phase A: projection, X written to HBM in natural order --
            # DMA issue order matters (per-queue FIFO): head e prefetches
            # W[e+1] and yt[e+1]; Phase-B tables load after head 0.
            pa_ctx = tc.tile_pool(name="psumA",
                                  bufs=int(os.environ.get("ES_PABUFS", "4")),
                                  space="PSUM")
            pspool = pa_ctx.__enter__()
            yts = {}

            def load_head(e):
                if WSPLIT:
                    load_w(e)
                yts[e] = ypool.tile([128, K], mdt, tag="yt", name=f"yt{e}")
                nc.sync.dma_start(out=yts[e][:], in_=yt[:, e * K:(e + 1) * K])

            if "A" in PHASES:
                load_head(0)
                if "B" in PHASES:
                    nc.sync.dma_start(out=gidx_sb[:], in_=gidx[:])
                    nc.sync.dma_start(out=relc_sb[:], in_=relc[:])
                    nc.sync.dma_start(out=cols_sb[:], in_=cols[:])
                load_head(1)
                load_head(2)
            for e in range(HEADS if "A" in PHASES else 0):
                yt_e = yts.pop(e)
                if e + 3 < HEADS:
                    load_head(e + 3)
                for rc in range(K // 128):
                    px = pspool.tile([128, OUT_DIM], F32, tag="pa")
                    lhsT = yt_e[:, rc * 128:(rc + 1) * 128]
                    for h in range(2):
                        nc.tensor.matmul(
                            out=px[:, h * 512:(h + 1) * 512],
                            lhsT=lhsT,
                            rhs=w_sb[:, e * OUT_DIM + h * 512:
                                     e * OUT_DIM + (h + 1) * 512],
                            start=True, stop=True,
                        )
                    xc = xpool.tile([128, OUT_DIM], sdt, tag="xc")
                    if ALT_COPY:
                        nc.vector.tensor_copy(out=xc[:, :512],
                                              in_=px[:, :512])
                        nc.scalar.copy(out=xc[:, 512:], in_=px[:, 512:])
                    else:
                        nc.vector.tensor_copy(out=xc[:], in_=px[:])
                    row0 = (e * (K // 128) + rc) * 128
                    xeng = (nc.scalar if os.environ.get("ES_DMAALT", "0") == "1"
                            and rc % 2 == 0 else nc.sync)
                    xeng.dma_start(out=xnat[row0:row0 + 128, :], in_=xc[:])
            if "A" not in PHASES and "B" in PHASES:
                nc.sync.dma_start(out=gidx_sb[:], in_=gidx[:])
                nc.sync.dma_start(out=relc_sb[:], in_=relc[:])
                nc.sync.dma_start(out=cols_sb[:], in_=cols[:])

            pa_ctx.__exit__(None, None, None)

            # Fence: every gather below reads rows written above.
            if BARRIER and "A" in PHASES and "B" in PHASES:
                tc.strict_bb_all_engine_barrier()
            pb_ctx = tc.tile_pool(name="psumB",
                                  bufs=int(os.environ.get("ES_PBBUFS", "2")),
                                  space="PSUM")
            pspool = pb_ctx.__enter__()

            # ---- Phase B: gather sorted rows per tile, one-hot matmul ----
            splitg = os.environ.get("ES_SPLITG", "1") == "1"
            gmin = min(gnums)
            prep0 = (os.environ.get("ES_PREP", "0") == "1"
                     and "A" in PHASES and "B" in PHASES and splitg
                     and not EXACT_CNT)
            g0 = None
            if prep0:
                # Prepare tile 0's gather descriptors during Phase A (the
                # Pool queue is idle); trigger_dma fires them the moment the
                # last xnat write lands, hiding desc-gen in the fence.
                gnum = gnums[0]
                cut = int(os.environ.get("ES_GCUT", "384"))
                ngt = -(-gnum // 128)
                g0 = gpool.tile([128, NG, OUT_DIM], sdt, tag="g", name="g0")
                if gmin < PT:
                    lastp = (gmin // 128) * 128
                    nc.gpsimd.memset(g0[gmin - lastp:, NG - 1, :], 0.0)
                sem0 = nc.alloc_semaphore("DMASW0")
                sem1 = nc.alloc_semaphore("DMASW1")
                nc.gpsimd.dma_gather(
                    out_ap=g0[:, 0:cut // 128, :], in_ap=xnat[:],
                    idxs_ap=gidx_sb[:, 0:cut // 16],
                    num_idxs=cut, num_idxs_reg=cut, elem_size=OUT_DIM,
                    prepare_only=True, sem=sem0,
                )
                nc.gpsimd.dma_gather(
                    out_ap=g0[:, cut // 128:ngt, :], in_ap=xnat[:],
                    idxs_ap=gidx_sb[:, cut // 16:gnum // 16],
                    num_idxs=gnum - cut, num_idxs_reg=gnum - cut,
                    elem_size=OUT_DIM, prepare_only=True, sem=sem1,
                )
                nc.gpsimd.trigger_dma(count=2)
            for t in range(NT if "B" in PHASES else 0):
                gnum = gnums[t]
                if prep0 and t == 0:
                    g = g0
                else:
                    g = gpool.tile([128, NG, OUT_DIM], sdt, tag="g")
                if prep0 and t == 0:
                    pass
                elif EXACT_CNT and t < GBUFS:
                    # With -1 skip-padding, unwritten positions vary per
                    # tile; scrub whole fresh slots once so unread regions
                    # hold finite values (one-hot sentinel zeroes them).
                    nc.gpsimd.memset(g[:], 0.0)
                elif gmin < PT and t < GBUFS:
                    # Positions gmin..PT may never be gathered; scrub the
                    # fresh SBUF slots once so the unread region holds
                    # finite values (one-hot sentinel zeroes them out).
                    lastp = (gmin // 128) * 128
                    nc.gpsimd.memset(g[gmin - lastp:, NG - 1, :], 0.0)
                gq = (t % 2) if os.environ.get("ES_GQALT", "0") == "1" else 0
                ngt = -(-gnum // 128)
                if prep0 and t == 0:
                    pass
                elif splitg:
                    cut = int(os.environ.get("ES_GCUT", "384"))
                    nc.gpsimd.dma_gather(
                        out_ap=g[:, 0:cut // 128, :],
                        in_ap=xnat[:],
                        idxs_ap=gidx_sb[:, t * (PT // 16):
                                        t * (PT // 16) + cut // 16],
                        num_idxs=cut, num_idxs_reg=cut, elem_size=OUT_DIM,
                        queue_num=gq,
                    )
                    nc.gpsimd.dma_gather(
                        out_ap=g[:, cut // 128:ngt, :],
                        in_ap=xnat[:],
                        idxs_ap=gidx_sb[:, t * (PT // 16) + cut // 16:
                                        t * (PT // 16) + gnum // 16],
                        num_idxs=gnum - cut, num_idxs_reg=gnum - cut,
                        elem_size=OUT_DIM, queue_num=gq,
                    )
                else:
                    nc.gpsimd.dma_gather(
                        out_ap=g[:],
                        in_ap=xnat[:],
                        idxs_ap=gidx_sb[:, t * (PT // 16):
                                        t * (PT // 16) + gnum // 16],
                        num_idxs=gnum,
                        num_idxs_reg=gnum,
                        elem_size=OUT_DIM,
                    )
                pt = pspool.tile([128, OUT_DIM], F32, tag="pb")
                for gi in range(ngt):
                    oh = ohpool.tile([128, 128], sdt, tag="oh")
                    c = t * NG + gi
                    nc.vector.tensor_tensor(
                        out=oh[:],
                        in0=relc_sb[:, c:c + 1].to_broadcast([128, 128]),
                        in1=cols_sb[:],
                        op=mybir.AluOpType.is_equal,
                    )
                    for h in range(2):
                        nc.tensor.matmul(
                            out=pt[:, h * 512:(h + 1) * 512],
                            lhsT=oh[:],
                            rhs=g[:, gi, h * 512:(h + 1) * 512],
                            start=(gi == 0), stop=(gi == ngt - 1),
                        )
                ot = opool.tile([128, OUT_DIM], odt, tag="ot")
                if ALT_COPY:
                    nc.vector.tensor_copy(out=ot[:, :512], in_=pt[:, :512])
                    nc.scalar.copy(out=ot[:, 512:], in_=pt[:, 512:])
                else:
                    nc.vector.tensor_copy(out=ot[:], in_=pt[:])
                nc.sync.dma_start(out=out[t * 128:(t + 1) * 128, :512],
                                  in_=ot[:, :512])
                nc.sync.dma_start(out=out[t * 128:(t + 1) * 128, 512:],
                                  in_=ot[:, 512:])
            pb_ctx.__exit__(None, None, None)

    nc.compile()
    return nc


def _get_program(gnums=(576,) * NT):
    if isinstance(gnums, int):
        gnums = (gnums,) * NT
    gnums = tuple(gnums)
    mdt = {"f32r": F32R, "f32": F32, "fp16": FP16, "bf16": BF16}[MM_DTYPE]
    sdt = {"f32r": F32R if MM_F32R else F32, "f32": F32,
           "bf16": BF16, "fp16": FP16}[X_DTYPE]
    key = (MM_DTYPE, X_DTYPE, PHASES, GBUFS, XBUFS, WSPLIT, BARRIER,
           ALT_COPY, EXACT_CNT, OUT_FP16, gnums,
           os.environ.get("ES_SPLITG", "1"),
           os.environ.get("ES_OBUFS", "4"), os.environ.get("ES_YBUFS", "4"),
           os.environ.get("ES_PABUFS", "2"), os.environ.get("ES_PBBUFS", "2"))
    if key not in _cache:
        _cache[key] = _build_program(mdt, sdt, gnums)
    return _cache[key]


def _prep_core_inputs(Yb, Indb):
    """Host-side prep for one batch: transpose Y, sort rows by slot,
    build padded gather-index and relative-column tables."""
    yt = np.ascontiguousarray(
        Yb.transpose(2, 0, 1).reshape(HEAD_DIM, R)).astype(np.float32)
    ind = Indb.reshape(R).astype(np.int64)
    order = np.argsort(ind, kind="stable")
    sind = ind[order]
    counts = np.bincount(sind // 128, minlength=NT)
    assert counts.max() <= PT, f"tile overflow: {counts.max()} > {PT}"
    _prep_core_inputs.tile_counts.append(counts)
    pad = -1 if EXACT_CNT else 0
    gidx = np.full((NT, PT), pad, dtype=np.int16)
    relc = np.full((NT, PT), -1000.0, dtype=np.float32)
    pos = 0
    for t in range(NT):
        c = counts[t]
        gidx[t, :c] = order[pos:pos + c]
        relc[t, :c] = (sind[pos:pos + c] - t * 128).astype(np.float32)
        pos += c
    # dma_gather index layout: position p -> (partition p%16, col p//16),
    # and the 16-partition block replicated across all 8 Q7 core groups.
    blk = np.concatenate(
        [gidx[t].reshape(PT // 16, 16).T for t in range(NT)], axis=1)
    gidx_sb = np.ascontiguousarray(np.tile(blk, (8, 1)), dtype=np.int16)
    # one-hot layout: position p -> (partition p%128, group p//128)
    relc_sb = np.concatenate(
        [relc[t].reshape(NG, 128).T for t in range(NT)], axis=1)
    relc_sb = np.ascontiguousarray(relc_sb, dtype=np.float32)
    return yt, gidx_sb, relc_sb


def kernel(Y, Ind, T, W):
    Y = np.asarray(Y, dtype=np.float32)
    Ind = np.asarray(Ind)
    W = np.asarray(W, dtype=np.float32)
    assert int(T) == T_SLOTS and Y.shape == (B, HEADS, K, HEAD_DIM)

    if MM_DTYPE == "fp16":
        np_mdt = np.float16
    elif MM_DTYPE == "bf16":
        import ml_dtypes
        np_mdt = ml_dtypes.bfloat16
    else:
        np_mdt = np.float32
    w_in = np.ascontiguousarray(
        W.transpose(1, 0, 2).reshape(HEAD_DIM, HEADS * OUT_DIM)
    ).astype(np_mdt)
    cols_in = np.broadcast_to(
        np.arange(128, dtype=np.float32)[None, :], (128, 128)).copy()

    _prep_core_inputs.tile_counts = []
    in_maps = []
    for b in range(B):
        yt, gidx_sb, relc_sb = _prep_core_inputs(Y[b], Ind[b])
        in_maps.append({
            "yt": yt.astype(np_mdt), "w": w_in, "gidx": gidx_sb,
            "relc": relc_sb, "cols": cols_in,
        })
    tmax = np.stack(_prep_core_inputs.tile_counts).max(axis=0)
    gnums = tuple(int(min(PT, max(512, -(-c // 16) * 16))) for c in tmax)
    nc = _get_program(gnums)

    # The first execution of a freshly compiled NEFF occasionally wedges a
    # core (NRT_EXEC_UNIT_UNRECOVERABLE); a retry on a fresh execute has
    # been observed to recover.
    last_exc = None
    for attempt in range(3):
        try:
            res = run_bass_kernel_spmd(
                nc, in_maps, core_ids=list(range(NCORES)),
                trace=os.environ.get("ES_TRACE", "0") == "1",
            )
            break
        except Exception as exc:  # noqa: BLE001 - device flake, retry
            last_exc = exc
            import time as _time
            _time.sleep(2.0)
    else:
        raise last_exc
    kernel.last_results = res
    out = np.stack([res.results[b]["out"] for b in range(B)], axis=0)
    return out.astype(np.float32)



# revision 21
# speedup vs baseline: 1.1415x; 1.0058x over previous
"""ExpertScatter TRN2 kernel.

reference semantics:
    X = einsum('bekj,eji->beki', Y, W)          # per-head projection
    out[b] = zeros([T, I]); out[b, Ind[b,e,k]] += X[b,e,k]

Strategy (data-parallel over batch, 1 batch per NeuronCore):
  Phase A: per head e, matmul X_chunk[128 rows, 1024] = Yt_chunk.T @ W[e]
           (fp16 operands), write X to an internal HBM staging buffer in
           natural row order (fp16 halves the round-trip traffic).
  Host precomputes a global sort of the 16384 rows of each batch by target
  slot. Sorted rows are gathered back in QUADS of output tiles (4 x 128
  slots): one padded gather per quad (padding = max over the 8 cores of the
  quad's row count, far less than per-tile padding). Pad positions index
  row 0 (finite data) and carry a -1000 relative-column sentinel.
  Phase B: per quad, dma_gather the contributing rows into SBUF; per output
  tile, build one-hot selection matrices on DVE (is_equal of the per-tile
  relative columns against a column-iota constant) for the tile's window of
  128-row groups, and accumulate out_tile = sum_g onehot_g.T @ Xrows_g in
  PSUM. Rows of neighboring tiles inside a shared group fall outside
  0..127 in relative-column space, so their one-hot coefficient is zero.
  Two DMAs per tile (512-col halves) write the output.

The program structure (quad paddings, per-tile group windows) is derived
from the actual per-core tile counts, so all 8 SPMD cores share one
program; per-core differences live in the input tensors (Yt, gather
indices, relative-column tables).
"""

import os

import numpy as np

import concourse.bacc as bacc
import concourse.mybir as mybir
import concourse.tile as tile
from concourse.bass_utils import run_bass_kernel_spmd

# Problem constants (hardcoded per harness contract).
B = 8
HEADS = 16
K = 1024
HEAD_DIM = 128
OUT_DIM = 1024
T_SLOTS = 4096

R = HEADS * K            # rows per batch = 16384
NT = T_SLOTS // 128      # output tiles per batch = 32
QT = 4                   # output tiles per gather quad
NQ = NT // QT            # quads per batch = 8
NCORES = 8
GPART = 768              # max gather rows per dma_gather (SWDGE ring safety)

F32 = mybir.dt.float32
F32R = mybir.dt.float32r
BF16 = mybir.dt.bfloat16
FP16 = mybir.dt.float16
I16 = mybir.dt.int16

MM_DTYPE = os.environ.get("ES_MM_DTYPE", "fp16")
X_DTYPE = os.environ.get("ES_X_DTYPE", "fp16")
PHASES = os.environ.get("ES_PHASES", "AB")
GBUFS = int(os.environ.get("ES_GBUFS", "2"))
XBUFS = int(os.environ.get("ES_XBUFS", "8"))
WSPLIT = os.environ.get("ES_WSPLIT", "1") == "1"
OUT_FP16 = os.environ.get("ES_OUT_FP16", "1") == "1"
ALT_COPY = os.environ.get("ES_ALT_COPY", "1") == "1"

_cache = {}


def _quad_meta(counts):
    """counts: [B, NT] per-core per-tile row counts -> static program meta.

    Returns (qpads, wlo, whi): per-quad padded gather sizes (x16), and each
    tile's window [wlo, whi) of 128-row groups inside its quad buffer.
    """
    counts = np.asarray(counts)
    qpads, wlo, whi = [], [], []
    for qi in range(NQ):
        sub = counts[:, qi * QT:(qi + 1) * QT]          # [B, QT]
        ends = np.cumsum(sub, axis=1)                   # [B, QT]
        starts = ends - sub
        qpads.append(int(-(-ends[:, -1].max() // 16) * 16))
        for t in range(QT):
            wlo.append(int(starts[:, t].min() // 128))
            whi.append(int(-(-ends[:, t].max() // 128)))
    return tuple(qpads), tuple(wlo), tuple(whi)


def _build_program(mdt, sdt, qpads, wlo, whi):
    qgs = [-(-p // 128) for p in qpads]      # buffer groups per quad
    qgmax = max(qgs)
    gcols = sum(p // 16 for p in qpads)      # gidx columns
    wcols = sum(whi[t] - wlo[t] for t in range(NT))  # relc columns

    nc = bacc.Bacc("TRN2", target_bir_lowering=False, debug=False,
                   num_devices=NCORES)

    yt = nc.dram_tensor("yt", [HEAD_DIM, R], mdt, kind="ExternalInput").ap()
    w = nc.dram_tensor("w", [HEAD_DIM, HEADS * OUT_DIM], mdt,
                       kind="ExternalInput").ap()
    gidx = nc.dram_tensor("gidx", [128, gcols], I16,
                          kind="ExternalInput").ap()
    relc = nc.dram_tensor("relc", [128, wcols], F32,
                          kind="ExternalInput").ap()
    cols = nc.dram_tensor("cols", [128, 128], F32, kind="ExternalInput").ap()
    odt = FP16 if OUT_FP16 else F32
    out = nc.dram_tensor("out", [T_SLOTS, OUT_DIM], odt,
                         kind="ExternalOutput").ap()
    xnat = nc.dram_tensor("xnat", [R, OUT_DIM], sdt).ap()

    with tile.TileContext(nc) as tc:
        with (
            tc.tile_pool(name="const", bufs=1) as cpool,
            tc.tile_pool(name="yhead",
                         bufs=int(os.environ.get("ES_YBUFS", "9"))) as ypool,
            tc.tile_pool(name="xchunk", bufs=XBUFS) as xpool,
            tc.tile_pool(name="gather", bufs=GBUFS) as gpool,
            tc.tile_pool(name="onehot",
                         bufs=int(os.environ.get("ES_OHBUFS", "4"))) as ohpool,
            tc.tile_pool(name="otile",
                         bufs=int(os.environ.get("ES_OBUFS", "4"))) as opool,
        ):
            w_sb = cpool.tile([128, HEADS * OUT_DIM], mdt, tag="w")
            gidx_sb = cpool.tile([128, gcols], I16, tag="gidx")
            relc_sb = cpool.tile([128, wcols], F32, tag="relc")
            cols_sb = cpool.tile([128, 128], F32, tag="cols")

            def load_w(e):
                nc.sync.dma_start(
                    out=w_sb[:, e * OUT_DIM:(e + 1) * OUT_DIM],
                    in_=w[:, e * OUT_DIM:(e + 1) * OUT_DIM])

            if not WSPLIT:
                nc.sync.dma_start(out=w_sb[:], in_=w[:])

            # ---- Phase A: projection, X written to HBM in natural order --
            # DMA issue order matters (per-queue FIFO): a few heads are
            # prefetched up front, the rest interleave one per head to keep
            # the DMA queue fed while the matmul/copy pipeline warms up.
            pa_ctx = tc.tile_pool(name="psumA",
                                  bufs=int(os.environ.get("ES_PABUFS", "4")),
                                  space="PSUM")
            pspool = pa_ctx.__enter__()
            yts = {}

            def load_head(e):
                if WSPLIT:
                    load_w(e)
                yts[e] = ypool.tile([128, K], mdt, tag="yt", name=f"yt{e}")
                nc.sync.dma_start(out=yts[e][:], in_=yt[:, e * K:(e + 1) * K])

            if "A" in PHASES:
                load_head(0)
                if "B" in PHASES:
                    nc.sync.dma_start(out=gidx_sb[:], in_=gidx[:])
                    nc.sync.dma_start(out=relc_sb[:], in_=relc[:])
                    nc.sync.dma_start(out=cols_sb[:], in_=cols[:])
                pf = int(os.environ.get("ES_PF", "4"))
                for ee in range(1, 1 + pf):
                    load_head(ee)
            else:
                pf = 0
            for e in range(HEADS if "A" in PHASES else 0):
                yt_e = yts.pop(e)
                if e + pf + 1 < HEADS:
                    load_head(e + pf + 1)
                for rc in range(K // 128):
                    px = pspool.tile([128, OUT_DIM], F32, tag="pa")
                    lhsT = yt_e[:, rc * 128:(rc + 1) * 128]
                    for h in range(2):
                        nc.tensor.matmul(
                            out=px[:, h * 512:(h + 1) * 512],
                            lhsT=lhsT,
                            rhs=w_sb[:, e * OUT_DIM + h * 512:
                                     e * OUT_DIM + (h + 1) * 512],
                            start=True, stop=True,
                        )
                    xc = xpool.tile([128, OUT_DIM], sdt, tag="xc")
                    if ALT_COPY:
                        nc.vector.tensor_copy(out=xc[:, :512],
                                              in_=px[:, :512])
                        nc.scalar.copy(out=xc[:, 512:], in_=px[:, 512:])
                    else:
                        nc.vector.tensor_copy(out=xc[:], in_=px[:])
                    row0 = (e * (K // 128) + rc) * 128
                    nc.sync.dma_start(out=xnat[row0:row0 + 128, :], in_=xc[:])

            if "A" not in PHASES and "B" in PHASES:
                nc.sync.dma_start(out=gidx_sb[:], in_=gidx[:])
                nc.sync.dma_start(out=relc_sb[:], in_=relc[:])
                nc.sync.dma_start(out=cols_sb[:], in_=cols[:])

            pa_ctx.__exit__(None, None, None)

            pb_ctx = tc.tile_pool(name="psumB",
                                  bufs=int(os.environ.get("ES_PBBUFS", "3")),
                                  space="PSUM")
            pspool = pb_ctx.__enter__()

            # ---- Phase B: gather sorted rows per quad, one-hot matmul ----
            # Scrub the last groups of the fresh gather slots once (on the
            # Pool queue, which idles during Phase A): the tail positions
            # qpad..qg*128 of each quad are never gathered, and matmul rhs
            # garbage there would poison PSUM (0 x inf = NaN) even under a
            # zero one-hot coefficient.
            qgmin = min(qgs)
            gtiles = {}
            for qi in range(min(GBUFS, NQ) if "B" in PHASES else 0):
                gtiles[qi] = gpool.tile([128, qgmax, OUT_DIM], sdt, tag="g",
                                        name=f"gq{qi}")
                nc.gpsimd.memset(gtiles[qi][:, qgmin - 1:qgmax, :], 0.0)
            gc0 = 0   # running gidx column base
            wc0 = 0   # running relc column base
            for qi in range(NQ if "B" in PHASES else 0):
                qpad, qg = qpads[qi], qgs[qi]
                g = gtiles.pop(qi) if qi in gtiles else gpool.tile(
                    [128, qgmax, OUT_DIM], sdt, tag="g")
                pos = 0
                while pos < qpad:
                    n = min(GPART, qpad - pos)
                    assert pos % 128 == 0 and n % 16 == 0
                    g1 = min(qg, (pos + n + 127) // 128)
                    nc.gpsimd.dma_gather(
                        out_ap=g[:, pos // 128:g1, :],
                        in_ap=xnat[:],
                        idxs_ap=gidx_sb[:, gc0 + pos // 16:
                                        gc0 + (pos + n) // 16],
                        num_idxs=n, num_idxs_reg=n, elem_size=OUT_DIM,
                    )
                    pos += n
                gc0 += qpad // 16
                for tq in range(QT):
                    t = qi * QT + tq
                    lo, hi = wlo[t], whi[t]
                    pt = pspool.tile([128, OUT_DIM], F32, tag="pb")
                    for j, gg in enumerate(range(lo, hi)):
                        oh = ohpool.tile([128, 128], sdt, tag="oh")
                        c = wc0 + j
                        nc.vector.tensor_tensor(
                            out=oh[:],
                            in0=relc_sb[:, c:c + 1].to_broadcast([128, 128]),
                            in1=cols_sb[:],
                            op=mybir.AluOpType.is_equal,
                        )
                        for h in range(2):
                            nc.tensor.matmul(
                                out=pt[:, h * 512:(h + 1) * 512],
                                lhsT=oh[:],
                                rhs=g[:, gg, h * 512:(h + 1) * 512],
                                start=(j == 0), stop=(j == hi - lo - 1),
                            )
                    wc0 += hi - lo
                    if ALT_COPY:
                        ot0 = opool.tile([128, 512], odt, tag="ot0")
                        ot1 = opool.tile([128, 512], odt, tag="ot1")
                        nc.vector.tensor_copy(out=ot0[:], in_=pt[:, :512])
                        nc.scalar.copy(out=ot1[:], in_=pt[:, 512:])
                        nc.sync.dma_start(
                            out=out[t * 128:(t + 1) * 128, :512], in_=ot0[:])
                        nc.sync.dma_start(
                            out=out[t * 128:(t + 1) * 128, 512:], in_=ot1[:])
                    else:
                        ot = opool.tile([128, OUT_DIM], odt, tag="ot")
                        nc.vector.tensor_copy(out=ot[:], in_=pt[:])
                        nc.sync.dma_start(out=out[t * 128:(t + 1) * 128, :],
                                          in_=ot[:])
            pb_ctx.__exit__(None, None, None)

    nc.compile()
    return nc


def _get_program(qpads, wlo, whi):
    mdt = {"f32r": F32R, "f32": F32, "fp16": FP16, "bf16": BF16}[MM_DTYPE]
    sdt = {"f32r": F32, "f32": F32, "bf16": BF16, "fp16": FP16}[X_DTYPE]
    key = (MM_DTYPE, X_DTYPE, PHASES, GBUFS, XBUFS, WSPLIT,
           ALT_COPY, OUT_FP16, qpads, wlo, whi,
           os.environ.get("ES_OBUFS", "4"), os.environ.get("ES_YBUFS", "9"),
           os.environ.get("ES_PABUFS", "4"), os.environ.get("ES_PBBUFS", "3"),
           os.environ.get("ES_PF", "4"))
    if key not in _cache:
        _cache[key] = _build_program(mdt, sdt, qpads, wlo, whi)
    return _cache[key]


def _count_tiles(Indb):
    ind = Indb.reshape(R).astype(np.int64)
    return np.bincount(ind // 128, minlength=NT)


def _prep_core_inputs(Yb, Indb, qpads, wlo, whi):
    """Host-side prep for one batch: transpose Y, sort rows by slot, build
    the quad gather-index and per-tile-window relative-column tables."""
    yt = np.ascontiguousarray(
        Yb.transpose(2, 0, 1).reshape(HEAD_DIM, R)).astype(np.float32)
    ind = Indb.reshape(R).astype(np.int64)
    order = np.argsort(ind, kind="stable")
    sind = ind[order]
    # quad boundaries in sorted position space
    qend = np.searchsorted(sind, [(qi + 1) * QT * 128 for qi in range(NQ)])
    qstart = np.concatenate([[0], qend[:-1]])

    gidx_blocks = []
    relc_cols = []
    for qi in range(NQ):
        qpad = qpads[qi]
        s, e = int(qstart[qi]), int(qend[qi])
        cntq = e - s
        assert cntq <= qpad, f"quad overflow: {cntq} > {qpad}"
        rows = np.zeros(qpad, dtype=np.int16)
        rows[:cntq] = order[s:e]
        gidx_blocks.append(rows.reshape(qpad // 16, 16).T)
        rel = np.full(qpad, -100000.0, dtype=np.float64)
        rel[:cntq] = sind[s:e].astype(np.float64)
        for tq in range(QT):
            t = qi * QT + tq
            for gg in range(wlo[t], whi[t]):
                col = np.full(128, -1000.0, dtype=np.float32)
                seg = rel[gg * 128:(gg + 1) * 128] - t * 128
                col[:len(seg)] = np.where(
                    (seg >= 0) & (seg < 128), seg, -1000.0)
                relc_cols.append(col.astype(np.float32))
    blk = np.concatenate(gidx_blocks, axis=1)
    gidx_sb = np.ascontiguousarray(np.tile(blk, (8, 1)), dtype=np.int16)
    relc_sb = np.ascontiguousarray(np.stack(relc_cols, axis=1),
                                   dtype=np.float32)
    return yt, gidx_sb, relc_sb


def kernel(Y, Ind, T, W):
    Y = np.asarray(Y, dtype=np.float32)
    Ind = np.asarray(Ind)
    W = np.asarray(W, dtype=np.float32)
    assert int(T) == T_SLOTS and Y.shape == (B, HEADS, K, HEAD_DIM)

    if MM_DTYPE == "fp16":
        np_mdt = np.float16
    elif MM_DTYPE == "bf16":
        import ml_dtypes
        np_mdt = ml_dtypes.bfloat16
    else:
        np_mdt = np.float32
    w_in = np.ascontiguousarray(
        W.transpose(1, 0, 2).reshape(HEAD_DIM, HEADS * OUT_DIM)
    ).astype(np_mdt)
    cols_in = np.broadcast_to(
        np.arange(128, dtype=np.float32)[None, :], (128, 128)).copy()

    counts = np.stack([_count_tiles(Ind[b]) for b in range(B)])
    qpads, wlo, whi = _quad_meta(counts)
    nc = _get_program(qpads, wlo, whi)

    in_maps = []
    for b in range(B):
        yt, gidx_sb, relc_sb = _prep_core_inputs(Y[b], Ind[b],
                                                 qpads, wlo, whi)
        in_maps.append({
            "yt": yt.astype(np_mdt), "w": w_in, "gidx": gidx_sb,
            "relc": relc_sb, "cols": cols_in,
        })

    # The first execution of a freshly compiled NEFF occasionally wedges a
    # core (NRT_EXEC_UNIT_UNRECOVERABLE); a retry on a fresh execute has
    # been observed to recover.
    last_exc = None
    for attempt in range(3):
        try:
            res = run_bass_kernel_spmd(
                nc, in_maps, core_ids=list(range(NCORES)),
                trace=os.environ.get("ES_TRACE", "0") == "1",
            )
            break
        except Exception as exc:  # noqa: BLE001 - device flake, retry
            last_exc = exc
            import time as _time
            _time.sleep(2.0)
    else:
        raise last_exc
    kernel.last_results = res
    out = np.stack([res.results[b]["out"] for b in range(B)], axis=0)
    return out.astype(np.float32)
